# revision 8
# baseline (speedup 1.0000x reference)
"""Trainium2 Bass kernel for nn_DHT_Layer (conv1x1+BN+ReLU -> Deep Hough
Transform -> two 3x3 conv+BN+ReLU layers).

Sharding: data-parallel over batch. 8 images / 8 cores -> one image per core,
no collectives; full inputs in, full output out. Inside each core:
  conv1   : 1x1 conv as ONE fp8 DoubleRow matmul per 500-col chunk (the two
            128-channel K-halves ride the DoubleRow pair dim), BN+ReLU folded
            into the scalar-engine epilogue (x and w1 quantized to e4m3;
            the DHT averaging washes the quantization out).
  DHT     : out[c,a,r] = sum_p h[c,p] * (idx[a,p]==r) as windowed one-hot
            matmuls (see baseline docstring for the windowing argument), now
            in fp8 DoubleRow: adjacent 128-pixel chunks pair up on the
            DoubleRow K dim (K=256 per instruction at 0.5 cyc/col), so the
            streamed column count drops ~4x vs the bf16 one-hot.  The one-hot
            is exact in e4m3 (0/1).  All one-hot volume is DMA-streamed as a
            pre-expanded fp8 table (DVE freed entirely; the shared-DMA-engine
            budget has room).
  conv2/3 : bf16 (fp8 here would land right at the 2e-2 error gate), as 9
            shifted matmuls over a zero-padded [c, 102*102] layout.

Cost-model device time: see test.py output.

The local walrus build only supports ONE sync-wait per instruction, so a
post-pass splits multi-wait instructions into single-wait NoOp carriers.
"""

import functools
import math

import ml_dtypes
import numpy as np

N = 8          # batch / cores
CIN = 256
CMID = 128
H = W = 100
HW = H * W
A = 100        # angles
R = 100        # rho bins
P = 128
NCHUNK = (HW + P - 1) // P    # 79 pixel chunks of 128
NCHUNK_PAD = NCHUNK + 1       # pad to even for DoubleRow chunk pairing
NPAIR = NCHUNK_PAD // 2       # 40
TAIL = HW - (NCHUNK - 1) * P  # 16 valid pixels in last real chunk
PADW = W + 2                  # 102 padded spatial for 3x3 convs
BN_EPS = 1e-5
GSIZE = 4      # angles per group (one PSUM bank, 4 slots of 128)
SLOT = 128
W1SCALE = 8.0  # fp8 range scaling for conv1 weights (undone in epilogue)
BF16 = ml_dtypes.bfloat16
F8 = ml_dtypes.float8_e4m3


# ----------------------------------------------------------------------------
# host-side precomputation (shapes are fixed -> cache)
# ----------------------------------------------------------------------------

def _hough_idx():
    irho = int(math.sqrt(H * H + W * W) + 1) / float(R)
    theta = np.arange(A) * (math.pi / A)
    tab_cos = np.cos(theta) / irho
    tab_sin = np.sin(theta) / irho
    yy, xx = np.meshgrid(np.arange(H) - H // 2, np.arange(W) - W // 2,
                         indexing='ij')
    xxf = xx.reshape(-1).astype(np.float64)
    yyf = yy.reshape(-1).astype(np.float64)
    r = np.round(xxf[None, :] * tab_cos[:, None] + yyf[None, :] * tab_sin[:, None])
    idx = np.clip((r + R // 2).astype(np.int32), 0, R - 1)  # [A, HW] row-major
    return idx, tab_cos, tab_sin


def _consecutive_runs(vals):
    runs = []
    cur = [vals[0]]
    for v in vals[1:]:
        if v == cur[-1] + 1:
            cur.append(v)
        else:
            runs.append(cur)
            cur = [v]
    runs.append(cur)
    return runs


@functools.lru_cache(maxsize=1)
def _dht_tables():
    idx, tab_cos, tab_sin = _hough_idx()
    # row-major contraction (pixels advance along x) is narrow when |cos| small
    rm_mask = np.abs(tab_cos) <= np.abs(tab_sin)
    idx_cm = idx.reshape(A, H, W).transpose(0, 2, 1).reshape(A, HW)

    groups = []
    for layout in ('rm', 'cm'):
        alist = [a for a in range(A) if (rm_mask[a] if layout == 'rm' else not rm_mask[a])]
        for run in _consecutive_runs(alist):
            for i in range(0, len(run), GSIZE):
                g_angles = run[i:i + GSIZE]
                src = idx if layout == 'rm' else idx_cm
                gidx = src[g_angles]                      # [g, HW]
                # relative index per padded chunk, -1 for invalid pixels
                vals = np.full((len(g_angles), NCHUNK_PAD * P), -1.0)
                vals[:, :HW] = gidx
                gc = vals.reshape(len(g_angles), NCHUNK_PAD, P)
                # per-PAIR window over VALID pixels of both half-chunks
                gp = gc.reshape(len(g_angles), NPAIR, 2 * P)
                lo = np.zeros(NPAIR, np.int32)
                hi = np.zeros(NPAIR, np.int32)
                for j in range(NPAIR):
                    v = gp[:, j, :]
                    vv = v[v >= 0]
                    if vv.size == 0:
                        lo[j], hi[j] = 0, 0
                    else:
                        lo[j], hi[j] = int(vv.min()), int(vv.max())
                win = int((hi - lo + 1).max())
                lo = np.minimum(lo, SLOT - win).astype(np.int32)
                groups.append(dict(layout=layout, angles=g_angles, win=win,
                                   lo=lo, a0=g_angles[0], rel=gc))

    # expanded fp8 one-hot table per group, layout per partition row:
    # col = ((j pair * 2 + h half) * win + r) * gl + i  -> [P, NPAIR*2*win*gl]
    tparts = []
    cursor = 0
    for g in groups:
        gl = len(g['angles'])
        win = g['win']
        gc = g['rel']                                   # [gl, NCHUNK_PAD, P]
        g['tbase'] = cursor
        onehot = np.zeros((P, NPAIR, 2, win, gl), np.float32)
        jr = np.arange(win)
        for ii in range(gl):
            v = gc[ii].reshape(NPAIR, 2, P)             # [j, h, P]
            rel = v - g['lo'][:, None, None]
            rel[v < 0] = -1.0
            onehot[:, :, :, :, ii] = (
                rel.transpose(2, 0, 1)[:, :, :, None] == jr[None, None, None, :win])
        tparts.append(onehot.reshape(P, NPAIR * 2 * win * gl))
        cursor += NPAIR * 2 * win * gl
        del g['rel']
    ohtable = np.ascontiguousarray(np.concatenate(tparts, 1).astype(F8))
    return dict(groups=groups, ohtable=ohtable)


def _prep_weights(w1, b1, g1, be1, m1, v1, w2, b2, g2, be2, m2, v2,
                  w3, b3, g3, be3, m3, v3):
    s1 = g1 / np.sqrt(v1 + BN_EPS)
    s2 = g2 / np.sqrt(v2 + BN_EPS)
    s3 = g3 / np.sqrt(v3 + BN_EPS)
    # conv1: y[co] = sum_ci w1[co,ci]*x[ci]; fold BN scale into co rows.
    # fp8: scale by W1SCALE into e4m3's sweet spot; epilogue divides it out.
    w1f = (w1[:, :, 0, 0] * s1[:, None]).T * W1SCALE  # [ci=256, co=128]
    w1p = np.ascontiguousarray(
        w1f.reshape(2, 128, 128).astype(F8))          # [half, ci128, co]
    bias1 = ((b1 - m1) * s1 + be1).astype(np.float32).reshape(128, 1)
    # conv2/3: [9 taps][ci, co], scaled by s[co]
    w2f = (w2 * s2[:, None, None, None]).transpose(2, 3, 1, 0)  # [ky,kx,ci,co]
    w2p = np.ascontiguousarray(w2f.reshape(9, 128, 128).astype(BF16))
    bias2 = ((b2 - m2) * s2 + be2).astype(np.float32).reshape(128, 1)
    w3f = (w3 * s3[:, None, None, None]).transpose(2, 3, 1, 0)
    w3p = np.ascontiguousarray(w3f.reshape(9, 128, 128).astype(BF16))
    bias3 = ((b3 - m3) * s3 + be3).astype(np.float32).reshape(128, 1)
    ident = np.eye(128, dtype=BF16)
    return w1p, bias1, w2p, bias2, w3p, bias3, ident


# ----------------------------------------------------------------------------
# walrus workaround: split multi-wait instructions (this build supports only
# one sync-wait per instruction)
# ----------------------------------------------------------------------------

def _split_multi_waits(nc, mybir, max_waits=1):
    cnt = 0
    for f in nc.m.functions:
        for bb in f.blocks:
            insts = list(bb.instructions)
            new = []
            changed = False
            for inst in insts:
                si = inst.sync_info
                if si is not None:
                    ow = list(si.on_wait)
                    if len(ow) > max_waits:
                        changed = True
                        head = ow[:-max_waits]
                        for i in range(0, len(head), max_waits):
                            nop = mybir.InstNoOp(name=f'waitsplit_{cnt}',
                                                 ins=[], outs=[])
                            cnt += 1
                            nop.engine = inst.engine
                            nop.sync_info = mybir.SyncInfo(
                                on_wait=head[i:i + max_waits], on_update=[])
                            new.append(nop)
                        si.on_wait = ow[-max_waits:]
                new.append(inst)
            if changed:
                bb.instructions = new
    return cnt


# ----------------------------------------------------------------------------
# bass program
# ----------------------------------------------------------------------------

_PROGRAM_CACHE = {}


def _build_program(split_waits=True, debug_outs=False, repeat=1):
    key = ('nc', split_waits, debug_outs, repeat)
    if key in _PROGRAM_CACHE:
        return _PROGRAM_CACHE[key]
    import concourse.bass as bass
    import concourse.mybir as mybir
    import concourse.tile as tile
    from contextlib import ExitStack

    T = _dht_tables()
    groups = T['groups']
    ohtable_cols = T['ohtable'].shape[1]

    f32 = mybir.dt.float32
    bf16 = mybir.dt.bfloat16
    f8 = mybir.dt.float8e4
    RELU = mybir.ActivationFunctionType.Relu
    COPY = mybir.ActivationFunctionType.Copy
    DR = mybir.MatmulPerfMode.DoubleRow

    nc = bass.Bass('TRN2', target_bir_lowering=False, debug=False)
    x_d = nc.dram_tensor('x', [CIN, HW], f8, kind='ExternalInput')
    w1_d = nc.dram_tensor('w1p', [2, 128, 128], f8, kind='ExternalInput')
    b1_d = nc.dram_tensor('bias1', [128, 1], f32, kind='ExternalInput')
    w2_d = nc.dram_tensor('w2p', [9, 128, 128], bf16, kind='ExternalInput')
    b2_d = nc.dram_tensor('bias2', [128, 1], f32, kind='ExternalInput')
    w3_d = nc.dram_tensor('w3p', [9, 128, 128], bf16, kind='ExternalInput')
    b3_d = nc.dram_tensor('bias3', [128, 1], f32, kind='ExternalInput')
    id_d = nc.dram_tensor('ident', [128, 128], bf16, kind='ExternalInput')
    tb_d = nc.dram_tensor('ohtable', [128, ohtable_cols], f8,
                          kind='ExternalInput')
    out_d = nc.dram_tensor('out', [128, HW], f32, kind='ExternalOutput')
    if debug_outs:
        dbg_h1_d = nc.dram_tensor('dbg_h1', [128, HW], bf16,
                                  kind='ExternalOutput')
        dbg_h1t_d = nc.dram_tensor('dbg_h1t', [128, NCHUNK_PAD * 128], f8,
                                   kind='ExternalOutput')
        dbg_dht_d = nc.dram_tensor('dbg_dht', [128, PADW * PADW], bf16,
                                   kind='ExternalOutput')
        dbg_h2_d = nc.dram_tensor('dbg_h2', [128, PADW * PADW], bf16,
                                  kind='ExternalOutput')

    with tile.TileContext(nc) as tc, ExitStack() as st0:
        consts = st0.enter_context(tc.tile_pool(name='consts', bufs=1))
        h1t_pool = st0.enter_context(tc.tile_pool(name='h1t', bufs=1))
        pad_pool = st0.enter_context(tc.tile_pool(name='pads', bufs=1))
        outb_pool = st0.enter_context(tc.tile_pool(name='outb', bufs=3))

        w1_t = consts.tile([128, 2 * 128], f8, tag='w1')
        nc.sync.dma_start(out=w1_t[:, 0:128], in_=w1_d.ap()[0])
        nc.sync.dma_start(out=w1_t[:, 128:256], in_=w1_d.ap()[1])
        w2_t = consts.tile([128, 9 * 128], bf16, tag='w2')
        w3_t = consts.tile([128, 9 * 128], bf16, tag='w3')
        b1_t = consts.tile([128, 1], f32, tag='b1')
        b2_t = consts.tile([128, 1], f32, tag='b2')
        b3_t = consts.tile([128, 1], f32, tag='b3')
        nc.sync.dma_start(out=b1_t[:], in_=b1_d.ap())
        nc.sync.dma_start(out=b2_t[:], in_=b2_d.ap())
        nc.sync.dma_start(out=b3_t[:], in_=b3_d.ap())
        id_t = consts.tile([128, 128], bf16, tag='ident')
        nc.sync.dma_start(out=id_t[:], in_=id_d.ap())
        zero_t = consts.tile([128, 1024], f8, tag='zeros')
        nc.vector.memset(zero_t[:], 0.0)

        h1T_rm = h1t_pool.tile([128, NCHUNK_PAD * 128], f8, tag='h1T_rm')
        h1T_cm = h1t_pool.tile([128, NCHUNK_PAD * 128], f8, tag='h1T_cm')
        # zero the tail chunk's stale rows + the pad chunk (garbage *
        # onehot-zero must be 0, and fp8 garbage could be NaN)
        nc.vector.memset(h1T_rm[:, (NCHUNK - 1) * 128:], 0.0)
        nc.vector.memset(h1T_cm[:, (NCHUNK - 1) * 128:], 0.0)

        dht_pad = pad_pool.tile([128, PADW * PADW], bf16, tag='dht_pad')
        h2_pad = pad_pool.tile([128, PADW * PADW], bf16, tag='h2_pad')
        # zero only the borders; the interior is fully overwritten
        for pad_t in (dht_pad, h2_pad):
            pv = pad_t[:].rearrange('c (a r) -> c a r', a=PADW)
            nc.gpsimd.memset(pv[:, 0:1, :], 0.0)
            nc.gpsimd.memset(pv[:, PADW - 1:PADW, :], 0.0)
            nc.gpsimd.memset(pv[:, :, 0:1], 0.0)
            nc.gpsimd.memset(pv[:, :, PADW - 1:PADW], 0.0)

        # ------------------------------------------------ pipeline body
        first_rep = [True]

        def emit_pipeline():
          with ExitStack() as stT:
            pst = stT.enter_context(
                tc.tile_pool(name='pst', bufs=2, space='PSUM'))
            h1_pool = stT.enter_context(tc.tile_pool(name='h1', bufs=1))
            # h1 stays bf16: the PE fp8-transpose path needs element-step-2
            # outputs (walrus restriction), so transposes run in bf16 and the
            # PSUM->SBUF copy does the fp8 conversion instead.
            h1 = h1_pool.tile([128, HW], bf16, tag='h1')
            h1cm = h1_pool.tile([128, HW], bf16, tag='h1cm')

            with ExitStack() as st1:
                xf_pool = st1.enter_context(tc.tile_pool(name='xf', bufs=6))
                ps1 = st1.enter_context(
                    tc.tile_pool(name='ps1', bufs=2, space='PSUM'))
                sizes = [500] * 20
                cs0 = 0
                for c, CS in enumerate(sizes):
                    sl = slice(cs0, cs0 + CS)
                    cs0 += CS
                    ps = ps1.tile([128, 500], f32, tag='ps1')
                    xf = xf_pool.tile([128, 2 * 500], f8, tag='xf')
                    for hh in range(2):
                        dma_eng = (nc.sync, nc.gpsimd,
                                   nc.scalar)[(2 * c + hh) % 3]
                        dma_eng.dma_start(
                            out=xf[:, hh * CS:(hh + 1) * CS],
                            in_=x_d.ap()[hh * 128:(hh + 1) * 128, sl])
                    nc.tensor.matmul(
                        out=ps[:, :CS],
                        lhsT=w1_t[:].rearrange('k (two m) -> k two m', two=2),
                        rhs=xf[:, :2 * CS].rearrange('k (two n) -> k two n',
                                                     two=2),
                        start=True, stop=True, perf_mode=DR)
                    nc.scalar.activation(out=h1[:, sl], in_=ps[:, :CS],
                                         func=RELU, bias=b1_t[:, :1],
                                         scale=1.0 / W1SCALE)

            if debug_outs:
                nc.sync.dma_start(out=dbg_h1_d.ap(), in_=h1[:])

            def transposes(src, dst, copy_eng):
                # 4 transposed chunks per PSUM tile -> one copy per 4
                for k0 in range(0, NCHUNK, 4):
                    kc = min(4, NCHUNK - k0)
                    pt = pst.tile([128, 512], bf16, tag='pt', space='PSUM')
                    for kk in range(kc):
                        k = k0 + kk
                        npx = TAIL if k == NCHUNK - 1 else 128
                        nc.tensor.transpose(
                            out=pt[:npx, kk * 128:(kk + 1) * 128],
                            in_=src[:, k * 128:k * 128 + npx],
                            identity=id_t[:])
                    if k0 + kc == NCHUNK:
                        # tail chunk: only TAIL partitions are valid; the
                        # memset zeros in dst rows TAIL.. must survive
                        if kc > 1:
                            copy_eng(
                                out=dst[:, k0 * 128:(k0 + kc - 1) * 128],
                                in_=pt[:, :(kc - 1) * 128])
                        copy_eng(
                            out=dst[:TAIL, (NCHUNK - 1) * 128:NCHUNK * 128],
                            in_=pt[:TAIL, (kc - 1) * 128:kc * 128])
                    else:
                        copy_eng(
                            out=dst[:, k0 * 128:(k0 + kc) * 128],
                            in_=pt[:, :kc * 128])

            def act_copy(out, in_):
                nc.scalar.copy(out=out, in_=in_)

            def dve_copy(out, in_):
                nc.vector.tensor_scalar_mul(out, in_, 1.0)

            transposes(h1, h1T_rm, act_copy)

            # ------------------------------------------ DHT
            with ExitStack() as st2:
                oh_pool = st2.enter_context(tc.tile_pool(name='oh', bufs=3))
                psd = st2.enter_context(
                    tc.tile_pool(name='psd', bufs=6, space='PSUM'))

                def emit_group(g, gi):
                    h1T = h1T_rm if g['layout'] == 'rm' else h1T_cm
                    gl = len(g['angles'])
                    win = g['win']
                    gcols = NPAIR * 2 * win * gl
                    oh = oh_pool.tile([128, NPAIR * 2 * 52 * GSIZE], f8,
                                      tag='oh')
                    deng = (nc.sync, nc.gpsimd)[gi % 2]
                    deng.dma_start(out=oh[:, :gcols],
                                   in_=tb_d.ap()[:, g['tbase']:
                                                 g['tbase'] + gcols])
                    pt = psd.tile([128, 512], f32, tag='psd', space='PSUM')
                    # zero + set has_written via K=1 zero DoubleRow matmul
                    nc.tensor.matmul(
                        out=pt[:],
                        lhsT=zero_t[:1, :256].rearrange(
                            'k (two m) -> k two m', two=2),
                        rhs=zero_t[:1, :1024].rearrange(
                            'k (two n) -> k two n', two=2),
                        start=True, stop=False, perf_mode=DR,
                        skip_group_check=True)
                    ohv = oh[:, :gcols].rearrange(
                        'p (j two n) -> p j two n', j=NPAIR, two=2)
                    for j in range(NPAIR):
                        lo = int(g['lo'][j])
                        nc.tensor.matmul(
                            out=pt[:, gl * lo:gl * (lo + win)],
                            lhsT=h1T[:, j * 256:(j + 1) * 256].rearrange(
                                'p (two m) -> p two m', two=2),
                            rhs=ohv[:, j],
                            start=False, stop=False, perf_mode=DR,
                            skip_group_check=True)
                    # copy accumulator into conv2 input (de-interleave)
                    a0 = g['a0']
                    pv = pt[:, :gl * 128].rearrange('p (r s) -> p s r', s=gl)
                    dv = dht_pad[:].rearrange('c (a r) -> c a r', a=PADW)
                    nc.scalar.activation(
                        out=dv[:, a0 + 1:a0 + 1 + gl, 1:1 + R],
                        in_=pv[:, :, :R], func=COPY)

                rm_groups = [g for g in groups if g['layout'] == 'rm']
                cm_groups = [g for g in groups if g['layout'] == 'cm']
                gi = 0
                for g in rm_groups:
                    emit_group(g, gi)
                    gi += 1
                # cm prep runs under the rm groups: the strided h1->h1cm copy
                # on DVE, then the cm transposes follow the rm matmul stream
                dve_copy(
                    out=h1cm[:],
                    in_=h1[:].rearrange('c (y x) -> c x y', y=H, x=W))
                transposes(h1cm, h1T_cm, dve_copy)
                if first_rep[0]:
                    first_rep[0] = False
                    for t9 in range(9):
                        nc.sync.dma_start(
                            out=w2_t[:, t9 * 128:(t9 + 1) * 128],
                            in_=w2_d.ap()[t9])
                        nc.sync.dma_start(
                            out=w3_t[:, t9 * 128:(t9 + 1) * 128],
                            in_=w3_d.ap()[t9])
                for g in cm_groups:
                    emit_group(g, gi)
                    gi += 1

                # ------------------------------------ conv2 / conv3
                # conv psum tiles share the DHT accumulator slots (same
                # tag), so conv2 chunks whose input rows are already
                # written (the rm-angle band) can fill PE gaps during the
                # cm passes.
                AR = 4  # angle rows per psum chunk
                for conv_i, (w_t, b_t, src_t) in enumerate(
                        ((w2_t, b2_t, dht_pad), (w3_t, b3_t, h2_pad))):
                    sv = src_t[:].rearrange('c (a r) -> c a r', a=PADW)
                    if conv_i == 0:
                        # rm-band chunks first: their input rows are done
                        # before the cm passes run, so they can fill PE gaps
                        corder = list(range(7, 18)) + \
                            [c for c in range(A // AR) if not 7 <= c < 18]
                    else:
                        corder = list(range(A // AR))
                    for c in corder:
                        a0 = c * AR
                        ps = psd.tile([128, 512], f32, tag='psd',
                                      space='PSUM')
                        for t9 in range(9):
                            dy, dx = divmod(t9, 3)
                            nc.tensor.matmul(
                                out=ps[:, :AR * R],
                                lhsT=w_t[:, t9 * 128:(t9 + 1) * 128],
                                rhs=sv[:, a0 + dy:a0 + dy + AR, dx:dx + R],
                                start=(t9 == 0), stop=(t9 == 8))
                        pv = ps[:, :AR * R].rearrange('p (a r) -> p a r',
                                                      a=AR)
                        if conv_i == 0:
                            hv = h2_pad[:].rearrange('c (a r) -> c a r',
                                                     a=PADW)
                            nc.scalar.activation(
                                out=hv[:, a0 + 1:a0 + 1 + AR, 1:1 + R],
                                in_=pv[:], func=RELU, bias=b_t[:, :1],
                                scale=1.0)
                            if debug_outs and c == A // AR - 1:
                                nc.sync.dma_start(out=dbg_h2_d.ap(),
                                                  in_=h2_pad[:])
                        else:
                            ob = outb_pool.tile([128, AR * R], f32,
                                                tag='outb')
                            ov = ob[:].rearrange('p (a r) -> p a r', a=AR)
                            nc.scalar.activation(out=ov[:], in_=pv[:],
                                                 func=RELU, bias=b_t[:, :1],
                                                 scale=1.0)
                            nc.sync.dma_start(
                                out=out_d.ap()[:, a0 * R:(a0 + AR) * R],
                                in_=ob[:])

          if debug_outs:
            nc.sync.dma_start(out=dbg_h1t_d.ap(), in_=h1T_rm[:])
            nc.sync.dma_start(out=dbg_dht_d.ap(), in_=dht_pad[:])

        for _rep in range(repeat):
            emit_pipeline()

    if split_waits:
        _split_multi_waits(nc, mybir)
    _PROGRAM_CACHE[key] = nc
    return nc


# ----------------------------------------------------------------------------
# entry point
# ----------------------------------------------------------------------------

def make_in_maps(inputs):
    T = _dht_tables()
    x = np.asarray(inputs['x'], np.float32)
    w1p, bias1, w2p, bias2, w3p, bias3, ident = _prep_weights(
        *[np.asarray(inputs[k], np.float32) for k in
          ('w1', 'b1', 'g1', 'be1', 'm1', 'v1',
           'w2', 'b2', 'g2', 'be2', 'm2', 'v2',
           'w3', 'b3', 'g3', 'be3', 'm3', 'v3')])
    common = dict(w1p=w1p, bias1=bias1, w2p=w2p, bias2=bias2, w3p=w3p,
                  bias3=bias3, ident=ident, ohtable=T['ohtable'])
    return [
        {'x': np.ascontiguousarray(x[n]).reshape(CIN, HW).astype(F8),
         **common}
        for n in range(N)
    ]


def run(inputs, trace=False):
    from concourse.bass_utils import run_bass_kernel_spmd

    nc = _build_program()
    in_maps = make_in_maps(inputs)
    res = run_bass_kernel_spmd(nc, in_maps, core_ids=list(range(N)),
                               trace=trace)
    out = np.stack([res.results[n]['out'].reshape(CMID, H, W)
                    for n in range(N)], axis=0)
    return out.astype(np.float32), res


def kernel(**inputs):
    out, _ = run(inputs, trace=False)
    return out


# revision 16
# speedup vs baseline: 1.0617x; 1.0617x over previous
"""Trainium2 Bass kernel for nn_DHT_Layer (conv1x1+BN+ReLU -> Deep Hough
Transform -> two 3x3 conv+BN+ReLU layers).

Sharding: data-parallel over batch. 8 images / 8 cores -> one image per core,
no collectives; full inputs in, full output out. Inside each core:
  conv1   : 1x1 conv computed DIRECTLY TRANSPOSED, twice (once per DHT pixel
            layout): stationary = x chunk [ci, px], moving = w1 [ci, co],
            out = [px, co] -> the BN+ReLU epilogue writes h1T straight to
            SBUF in fp8.  No PE transposes, no h1/h1cm intermediates.
            rm chunks are 128 contiguous row-major pixels; cm chunks are
            whole image columns (100 px, stride-100 views of x).
            fp8 DoubleRow: the two 128-channel K-halves ride the pair dim.
  DHT     : out[c,a,r] = sum_p h[c,p] * (idx[a,p]==r) as windowed one-hot
            matmuls in fp8 DoubleRow: adjacent pixel chunks pair up on the
            DoubleRow K dim (K=256/200 per instruction at 0.5 cyc/col).
            The one-hot is exact in e4m3 (0/1); all one-hot volume is
            DMA-streamed as a pre-expanded fp8 table over 3 queues
            (SP/Pool/DVE round-robin).
  conv2/3 : bf16 (fp8 here would land right at the 2e-2 error gate), as 9
            shifted matmuls over a zero-padded [c, 102*102] layout.
            conv2/conv3 chunks are interleaved into the DHT group stream as
            soon as their input angle-rows are complete, so the in-order PE
            queue always has DMA-independent work to absorb table-DMA
            latency.

The local walrus build only supports ONE sync-wait per instruction, so a
post-pass splits multi-wait instructions into single-wait NoOp carriers.
"""

import functools
import math

import ml_dtypes
import numpy as np

N = 8          # batch / cores
CIN = 256
CMID = 128
H = W = 100
HW = H * W
A = 100        # angles
R = 100        # rho bins
P = 128
NCH_RM = (HW + P - 1) // P    # 79 row-major pixel chunks of 128
NCH_RM_PAD = NCH_RM + 1       # pad to even for DoubleRow chunk pairing
NPAIR_RM = NCH_RM_PAD // 2    # 40
TAIL = HW - (NCH_RM - 1) * P  # 16 valid pixels in last real rm chunk
NCH_CM = W                    # 100 column chunks of 100 px
NPAIR_CM = NCH_CM // 2        # 50
PADW = W + 2                  # 102 padded spatial for 3x3 convs
BN_EPS = 1e-5
GSIZE = 4      # angles per group (one PSUM bank, 4 slots of 128)
SLOT = 128
W1SCALE = 8.0  # fp8 range scaling for conv1 weights (undone in epilogue)
BF16 = ml_dtypes.bfloat16
F8 = ml_dtypes.float8_e4m3


# ----------------------------------------------------------------------------
# host-side precomputation (shapes are fixed -> cache)
# ----------------------------------------------------------------------------

def _hough_idx():
    irho = int(math.sqrt(H * H + W * W) + 1) / float(R)
    theta = np.arange(A) * (math.pi / A)
    tab_cos = np.cos(theta) / irho
    tab_sin = np.sin(theta) / irho
    yy, xx = np.meshgrid(np.arange(H) - H // 2, np.arange(W) - W // 2,
                         indexing='ij')
    xxf = xx.reshape(-1).astype(np.float64)
    yyf = yy.reshape(-1).astype(np.float64)
    r = np.round(xxf[None, :] * tab_cos[:, None] + yyf[None, :] * tab_sin[:, None])
    idx = np.clip((r + R // 2).astype(np.int32), 0, R - 1)  # [A, HW] row-major
    return idx, tab_cos, tab_sin


def _consecutive_runs(vals):
    runs = []
    cur = [vals[0]]
    for v in vals[1:]:
        if v == cur[-1] + 1:
            cur.append(v)
        else:
            runs.append(cur)
            cur = [v]
    runs.append(cur)
    return runs


@functools.lru_cache(maxsize=1)
def _dht_tables():
    idx, tab_cos, tab_sin = _hough_idx()
    # row-major contraction (pixels advance along x) is narrow when |cos| small
    rm_mask = np.abs(tab_cos) <= np.abs(tab_sin)

    groups = []
    for layout in ('rm', 'cm'):
        alist = [a for a in range(A)
                 if (rm_mask[a] if layout == 'rm' else not rm_mask[a])]
        for run in _consecutive_runs(alist):
            for i in range(0, len(run), GSIZE):
                g_angles = run[i:i + GSIZE]
                gl = len(g_angles)
                if layout == 'rm':
                    # [gl, NCH_RM_PAD, P]: chunk k partition p = pixel k*128+p
                    vals = np.full((gl, NCH_RM_PAD * P), -1.0)
                    vals[:, :HW] = idx[g_angles]
                    gc = vals.reshape(gl, NCH_RM_PAD, P)
                    npair = NPAIR_RM
                else:
                    # [gl, NCH_CM, P]: chunk x0 partition y = pixel y*W+x0
                    # (rows 100..127 invalid)
                    gc = np.full((gl, NCH_CM, P), -1.0)
                    gidx = idx[g_angles].reshape(gl, H, W)  # [gl, y, x]
                    gc[:, :, :H] = gidx.transpose(0, 2, 1)  # [gl, x, y]
                    npair = NPAIR_CM
                gp = gc.reshape(gl, npair, 2 * P)
                lo = np.zeros(npair, np.int32)
                hi = np.zeros(npair, np.int32)
                for j in range(npair):
                    v = gp[:, j, :]
                    vv = v[v >= 0]
                    if vv.size == 0:
                        lo[j], hi[j] = 0, 0
                    else:
                        lo[j], hi[j] = int(vv.min()), int(vv.max())
                win = int((hi - lo + 1).max())
                lo = np.minimum(lo, SLOT - win).astype(np.int32)
                groups.append(dict(layout=layout, angles=g_angles, win=win,
                                   lo=lo, a0=g_angles[0], npair=npair,
                                   rel=gc))

    # expanded fp8 one-hot table per group, layout per partition row:
    # col = ((j pair * 2 + h half) * win + r) * gl + i  -> [P, npair*2*win*gl]
    tparts = []
    cursor = 0
    for g in groups:
        gl = len(g['angles'])
        win = g['win']
        npair = g['npair']
        gc = g['rel']                                   # [gl, nch, P]
        g['tbase'] = cursor
        onehot = np.zeros((P, npair, 2, win, gl), np.float32)
        jr = np.arange(win)
        for ii in range(gl):
            v = gc[ii].reshape(npair, 2, P)             # [j, h, P]
            rel = v - g['lo'][:, None, None]
            rel[v < 0] = -1.0
            onehot[:, :, :, :, ii] = (
                rel.transpose(2, 0, 1)[:, :, :, None] == jr[None, None, None, :win])
        tparts.append(onehot.reshape(P, npair * 2 * win * gl))
        cursor += npair * 2 * win * gl
        del g['rel']
    ohtable = np.ascontiguousarray(np.concatenate(tparts, 1).astype(F8))
    return dict(groups=groups, ohtable=ohtable)


def _prep_weights(w1, b1, g1, be1, m1, v1, w2, b2, g2, be2, m2, v2,
                  w3, b3, g3, be3, m3, v3):
    s1 = g1 / np.sqrt(v1 + BN_EPS)
    s2 = g2 / np.sqrt(v2 + BN_EPS)
    s3 = g3 / np.sqrt(v3 + BN_EPS)
    # conv1: y[co] = sum_ci w1[co,ci]*x[ci]; fold BN scale into co rows.
    # fp8: scale by W1SCALE into e4m3's sweet spot; epilogue divides it out.
    w1f = (w1[:, :, 0, 0] * s1[:, None]).T * W1SCALE  # [ci=256, co=128]
    w1p = np.ascontiguousarray(
        w1f.reshape(2, 128, 128).astype(F8))          # [half, ci128, co]
    bias1 = ((b1 - m1) * s1 + be1).astype(np.float32).reshape(128, 1)
    # conv2/3: [9 taps][ci, co], scaled by s[co]
    w2f = (w2 * s2[:, None, None, None]).transpose(2, 3, 1, 0)  # [ky,kx,ci,co]
    w2p = np.ascontiguousarray(w2f.reshape(9, 128, 128).astype(BF16))
    bias2 = ((b2 - m2) * s2 + be2).astype(np.float32).reshape(128, 1)
    w3f = (w3 * s3[:, None, None, None]).transpose(2, 3, 1, 0)
    w3p = np.ascontiguousarray(w3f.reshape(9, 128, 128).astype(BF16))
    bias3 = ((b3 - m3) * s3 + be3).astype(np.float32).reshape(128, 1)
    return w1p, bias1, w2p, bias2, w3p, bias3


# ----------------------------------------------------------------------------
# walrus workaround: split multi-wait instructions (this build supports only
# one sync-wait per instruction)
# ----------------------------------------------------------------------------

def _split_multi_waits(nc, mybir, max_waits=1):
    cnt = 0
    for f in nc.m.functions:
        for bb in f.blocks:
            insts = list(bb.instructions)
            new = []
            changed = False
            for inst in insts:
                si = inst.sync_info
                if si is not None:
                    ow = list(si.on_wait)
                    if len(ow) > max_waits:
                        changed = True
                        head = ow[:-max_waits]
                        for i in range(0, len(head), max_waits):
                            nop = mybir.InstNoOp(name=f'waitsplit_{cnt}',
                                                 ins=[], outs=[])
                            cnt += 1
                            nop.engine = inst.engine
                            nop.sync_info = mybir.SyncInfo(
                                on_wait=head[i:i + max_waits], on_update=[])
                            new.append(nop)
                        si.on_wait = ow[-max_waits:]
                new.append(inst)
            if changed:
                bb.instructions = new
    return cnt


# ----------------------------------------------------------------------------
# bass program
# ----------------------------------------------------------------------------

_PROGRAM_CACHE = {}


def _build_program(split_waits=True, debug_outs=False, repeat=1):
    key = ('nc', split_waits, debug_outs, repeat)
    if key in _PROGRAM_CACHE:
        return _PROGRAM_CACHE[key]
    import concourse.bass as bass
    import concourse.mybir as mybir
    import concourse.tile as tile
    from contextlib import ExitStack

    T = _dht_tables()
    groups = T['groups']
    ohtable_cols = T['ohtable'].shape[1]
    max_gcols = max(g['npair'] * 2 * g['win'] * len(g['angles'])
                    for g in groups)

    f32 = mybir.dt.float32
    bf16 = mybir.dt.bfloat16
    f8 = mybir.dt.float8e4
    RELU = mybir.ActivationFunctionType.Relu
    COPY = mybir.ActivationFunctionType.Copy
    DR = mybir.MatmulPerfMode.DoubleRow

    nc = bass.Bass('TRN2', target_bir_lowering=False, debug=False)
    x_d = nc.dram_tensor('x', [CIN, HW], f8, kind='ExternalInput')
    w1_d = nc.dram_tensor('w1p', [2, 128, 128], f8, kind='ExternalInput')
    # conv1 bias enters via a K=1 psum-prefill matmul (the direct-transposed
    # layout has channels on the FREE dim, out of reach of ACT's per-partition
    # bias): row0 = tile(W1SCALE*bias1, 4) ++ zeros(512), in fp8
    b1f_d = nc.dram_tensor('bias1f8', [1, 1024], f8, kind='ExternalInput')
    w2_d = nc.dram_tensor('w2p', [9, 128, 128], bf16, kind='ExternalInput')
    b2_d = nc.dram_tensor('bias2', [128, 1], f32, kind='ExternalInput')
    w3_d = nc.dram_tensor('w3p', [9, 128, 128], bf16, kind='ExternalInput')
    b3_d = nc.dram_tensor('bias3', [128, 1], f32, kind='ExternalInput')
    tb_d = nc.dram_tensor('ohtable', [128, ohtable_cols], f8,
                          kind='ExternalInput')
    out_d = nc.dram_tensor('out', [128, HW], f32, kind='ExternalOutput')
    if debug_outs:
        dbg_h1t_d = nc.dram_tensor('dbg_h1t', [128, NCH_RM_PAD * 128], f8,
                                   kind='ExternalOutput')
        dbg_dht_d = nc.dram_tensor('dbg_dht', [128, PADW * PADW], bf16,
                                   kind='ExternalOutput')
        dbg_h2_d = nc.dram_tensor('dbg_h2', [128, PADW * PADW], bf16,
                                  kind='ExternalOutput')

    with tile.TileContext(nc) as tc, ExitStack() as st0:
        consts = st0.enter_context(tc.tile_pool(name='consts', bufs=1))
        h1t_pool = st0.enter_context(tc.tile_pool(name='h1t', bufs=1))
        pad_pool = st0.enter_context(tc.tile_pool(name='pads', bufs=1))
        outb_pool = st0.enter_context(tc.tile_pool(name='outb', bufs=3))

        # full x resident: [128 part, (ci-half, y, x)] fp8 = 20 KB/partition
        x_t = consts.tile([128, 2 * HW], f8, tag='x')
        nc.sync.dma_start(out=x_t[:, 0:HW], in_=x_d.ap()[0:128, :])
        nc.gpsimd.dma_start(out=x_t[:, HW:2 * HW], in_=x_d.ap()[128:256, :])

        w1_t = consts.tile([128, 2 * 128], f8, tag='w1')
        nc.sync.dma_start(out=w1_t[:, 0:128], in_=w1_d.ap()[0])
        nc.sync.dma_start(out=w1_t[:, 128:256], in_=w1_d.ap()[1])
        w2_t = consts.tile([128, 9 * 128], bf16, tag='w2')
        w3_t = consts.tile([128, 9 * 128], bf16, tag='w3')
        b2_t = consts.tile([128, 1], f32, tag='b2')
        b3_t = consts.tile([128, 1], f32, tag='b3')
        nc.sync.dma_start(out=b2_t[:], in_=b2_d.ap())
        nc.sync.dma_start(out=b3_t[:], in_=b3_d.ap())
        zero_t = consts.tile([128, 1024], f8, tag='zeros')
        nc.vector.memset(zero_t[:], 0.0)
        # ones ++ zeros (lhsT of the bias prefill), bias row (its rhs)
        onez_t = consts.tile([1, 256], f8, tag='onez')
        nc.vector.memset(onez_t[:, 0:128], 1.0)
        nc.vector.memset(onez_t[:, 128:256], 0.0)
        b1f_t = consts.tile([1, 1024], f8, tag='b1f')
        nc.sync.dma_start(out=b1f_t[:], in_=b1f_d.ap())

        h1T_rm = h1t_pool.tile([128, NCH_RM_PAD * 128], f8, tag='h1T_rm')
        h1T_cm = h1t_pool.tile([128, NCH_CM * 128], f8, tag='h1T_cm')
        # zero the tail chunk's stale rows + the pad chunk (garbage *
        # onehot-zero must be 0, and fp8 garbage could be NaN)
        nc.vector.memset(h1T_rm[:, (NCH_RM - 1) * 128:], 0.0)

        dht_pad = pad_pool.tile([128, PADW * PADW], bf16, tag='dht_pad')
        h2_pad = pad_pool.tile([128, PADW * PADW], bf16, tag='h2_pad')
        # zero only the borders; the interior is fully overwritten
        for pad_t in (dht_pad, h2_pad):
            pv = pad_t[:].rearrange('c (a r) -> c a r', a=PADW)
            nc.gpsimd.memset(pv[:, 0:1, :], 0.0)
            nc.gpsimd.memset(pv[:, PADW - 1:PADW, :], 0.0)
            nc.gpsimd.memset(pv[:, :, 0:1], 0.0)
            nc.gpsimd.memset(pv[:, :, PADW - 1:PADW], 0.0)

        first_rep = [True]

        def emit_pipeline():
          with ExitStack() as stT:
            # w1 view [ci, 2, co]; x views [ci, 2, ...]
            w1v = w1_t[:].rearrange('k (two m) -> k two m', two=2)
            x_rm = x_t[:].rearrange('p (two q) -> p two q', two=2)
            x_cm = x_t[:].rearrange('p (two y x) -> p two x y', two=2, y=H)

            with ExitStack() as st1:
                ps1 = st1.enter_context(
                    tc.tile_pool(name='ps1', bufs=3, space='PSUM'))

                onez_v = onez_t[:].rearrange('k (two m) -> k two m', two=2)
                b1f_v = b1f_t[:].rearrange('k (two n) -> k two n', two=2)

                def bias_fill(pt):
                    # K=1 DoubleRow: out[p, kk*128+c] = bias[c] everywhere,
                    # also zeroes/claims the whole psum bank
                    nc.tensor.matmul(out=pt[:], lhsT=onez_v, rhs=b1f_v,
                                     start=True, stop=False, perf_mode=DR,
                                     skip_group_check=True)

                # ---- conv1 direct-transposed, rm chunks (128 px) ----
                for k0 in range(0, NCH_RM, 4):
                    kc = min(4, NCH_RM - k0)
                    pt = ps1.tile([128, 512], f32, tag='ps1', space='PSUM')
                    bias_fill(pt)
                    for kk in range(kc):
                        k = k0 + kk
                        npx = TAIL if k == NCH_RM - 1 else 128
                        nc.tensor.matmul(
                            out=pt[:npx, kk * 128:(kk + 1) * 128],
                            lhsT=x_rm[:, :, k * 128:k * 128 + npx],
                            rhs=w1v, start=False, stop=False,
                            perf_mode=DR, skip_group_check=True)
                    if k0 + kc == NCH_RM:
                        # tail: only TAIL h1T rows are valid for the last
                        # chunk; the memset zeros in h1T must survive
                        if kc > 1:
                            nc.scalar.activation(
                                out=h1T_rm[:, k0 * 128:(k0 + kc - 1) * 128],
                                in_=pt[:, :(kc - 1) * 128], func=RELU,
                                scale=1.0 / W1SCALE)
                        nc.scalar.activation(
                            out=h1T_rm[:TAIL,
                                       (NCH_RM - 1) * 128:NCH_RM * 128],
                            in_=pt[:TAIL, (kc - 1) * 128:kc * 128],
                            func=RELU, scale=1.0 / W1SCALE)
                    else:
                        nc.scalar.activation(
                            out=h1T_rm[:, k0 * 128:(k0 + kc) * 128],
                            in_=pt[:, :kc * 128], func=RELU,
                            scale=1.0 / W1SCALE)

                # ---- conv1 direct-transposed, cm chunks (100 px cols) ----
                for k0 in range(0, NCH_CM, 4):
                    pt = ps1.tile([128, 512], f32, tag='ps1', space='PSUM')
                    bias_fill(pt)
                    for kk in range(4):
                        nc.tensor.matmul(
                            out=pt[:H, kk * 128:(kk + 1) * 128],
                            lhsT=x_cm[:, :, k0 + kk, :],
                            rhs=w1v, start=False, stop=False,
                            perf_mode=DR, skip_group_check=True)
                    # fused scale + relu on DVE: max(in/8, 0)
                    nc.vector.tensor_scalar(
                        out=h1T_cm[:H, k0 * 128:(k0 + 4) * 128],
                        in0=pt[:H, :512], scalar1=1.0 / W1SCALE,
                        scalar2=0.0, op0=mybir.AluOpType.mult,
                        op1=mybir.AluOpType.max)

            if debug_outs:
                nc.sync.dma_start(out=dbg_h1t_d.ap(), in_=h1T_rm[:])

            # ------------------------------------------ DHT + convs
            with ExitStack() as st2:
                oh_pool = st2.enter_context(tc.tile_pool(name='oh', bufs=3))
                psd = st2.enter_context(
                    tc.tile_pool(name='psd', bufs=6, space='PSUM'))

                dma_rr = [0]
                dma_engs = (nc.sync, nc.gpsimd)

                def emit_group(g):
                    gl = len(g['angles'])
                    win = g['win']
                    npair = g['npair']
                    K = 128 if g['layout'] == 'rm' else H
                    h1T = h1T_rm if g['layout'] == 'rm' else h1T_cm
                    gcols = npair * 2 * win * gl
                    oh = oh_pool.tile([128, max_gcols], f8, tag='oh')
                    deng = dma_engs[dma_rr[0] % 2]
                    dma_rr[0] += 1
                    deng.dma_start(out=oh[:, :gcols],
                                   in_=tb_d.ap()[:, g['tbase']:
                                                 g['tbase'] + gcols])
                    pt = psd.tile([128, 512], f32, tag='psd', space='PSUM')
                    # zero + set has_written via K=1 zero DoubleRow matmul
                    nc.tensor.matmul(
                        out=pt[:],
                        lhsT=zero_t[:1, :256].rearrange(
                            'k (two m) -> k two m', two=2),
                        rhs=zero_t[:1, :1024].rearrange(
                            'k (two n) -> k two n', two=2),
                        start=True, stop=False, perf_mode=DR,
                        skip_group_check=True)
                    ohv = oh[:, :gcols].rearrange(
                        'p (j two n) -> p j two n', j=npair, two=2)
                    for j in range(npair):
                        lo = int(g['lo'][j])
                        nc.tensor.matmul(
                            out=pt[:, gl * lo:gl * (lo + win)],
                            lhsT=h1T[:K, j * 256:(j + 1) * 256].rearrange(
                                'p (two m) -> p two m', two=2),
                            rhs=ohv[:K, j],
                            start=False, stop=False, perf_mode=DR,
                            skip_group_check=True)
                    # copy accumulator into conv2 input (de-interleave) on
                    # DVE -- ACT is loaded with the conv epilogues
                    a0 = g['a0']
                    pv = pt[:, :gl * 128].rearrange('p (r s) -> p s r', s=gl)
                    dv = dht_pad[:].rearrange('c (a r) -> c a r', a=PADW)
                    nc.vector.tensor_scalar_mul(
                        dv[:, a0 + 1:a0 + 1 + gl, 1:1 + R],
                        pv[:, :, :R], 1.0)

                AR = 4   # angle rows per conv psum chunk
                NCC = A // AR

                def emit_conv(conv_i, c):
                    w_t, b_t, src_t = ((w2_t, b2_t, dht_pad),
                                       (w3_t, b3_t, h2_pad))[conv_i]
                    sv = src_t[:].rearrange('c (a r) -> c a r', a=PADW)
                    a0 = c * AR
                    ps = psd.tile([128, 512], f32, tag='psd', space='PSUM')
                    for t9 in range(9):
                        dy, dx = divmod(t9, 3)
                        nc.tensor.matmul(
                            out=ps[:, :AR * R],
                            lhsT=w_t[:, t9 * 128:(t9 + 1) * 128],
                            rhs=sv[:, a0 + dy:a0 + dy + AR, dx:dx + R],
                            start=(t9 == 0), stop=(t9 == 8))
                    pv = ps[:, :AR * R].rearrange('p (a r) -> p a r', a=AR)
                    if conv_i == 0:
                        hv = h2_pad[:].rearrange('c (a r) -> c a r', a=PADW)
                        nc.scalar.activation(
                            out=hv[:, a0 + 1:a0 + 1 + AR, 1:1 + R],
                            in_=pv[:], func=RELU, bias=b_t[:, :1], scale=1.0)
                    else:
                        ob = outb_pool.tile([128, AR * R], f32, tag='outb')
                        ov = ob[:].rearrange('p (a r) -> p a r', a=AR)
                        nc.scalar.activation(out=ov[:], in_=pv[:], func=RELU,
                                             bias=b_t[:, :1], scale=1.0)
                        deng = (nc.sync, nc.gpsimd)[c % 2]
                        deng.dma_start(
                            out=out_d.ap()[:, a0 * R:(a0 + AR) * R],
                            in_=ob[:])

                # ---- interleave DHT groups with conv2/conv3 chunks ----
                angles_done = set()
                conv2_done = set()
                conv3_done = set()

                def conv2_ready(c):
                    need = range(max(0, c * AR - 1), min(A, c * AR + AR + 1))
                    return all(a in angles_done for a in need)

                def conv3_ready(c):
                    need = range(max(0, c - 1), min(NCC, c + 2))
                    return all(cc in conv2_done for cc in need)

                def try_convs(budget):
                    done = 0
                    while done < budget:
                        c2 = next((c for c in range(NCC)
                                   if c not in conv2_done and conv2_ready(c)),
                                  None)
                        if c2 is not None:
                            emit_conv(0, c2)
                            conv2_done.add(c2)
                            done += 1
                            continue
                        c3 = next((c for c in range(NCC)
                                   if c not in conv3_done and conv3_ready(c)),
                                  None)
                        if c3 is not None:
                            emit_conv(1, c3)
                            conv3_done.add(c3)
                            done += 1
                            continue
                        break

                rm_groups = [g for g in groups if g['layout'] == 'rm']
                cm_groups = [g for g in groups if g['layout'] == 'cm']
                # alternate low/high cm runs to unlock conv2 edges evenly
                lo_cm = [g for g in cm_groups if g['a0'] < 50]
                hi_cm = [g for g in cm_groups if g['a0'] >= 50]
                cm_sched = []
                for i in range(max(len(lo_cm), len(hi_cm))):
                    if i < len(lo_cm):
                        cm_sched.append(lo_cm[i])
                    if i < len(hi_cm):
                        cm_sched.append(hi_cm[i])

                for gi, g in enumerate(rm_groups):
                    emit_group(g)
                    angles_done.update(g['angles'])
                    if first_rep[0] and gi == 1:
                        first_rep[0] = False
                        for t9 in range(9):
                            nc.sync.dma_start(
                                out=w2_t[:, t9 * 128:(t9 + 1) * 128],
                                in_=w2_d.ap()[t9])
                            nc.sync.dma_start(
                                out=w3_t[:, t9 * 128:(t9 + 1) * 128],
                                in_=w3_d.ap()[t9])
                    try_convs(2)
                for g in cm_sched:
                    emit_group(g)
                    angles_done.update(g['angles'])
                    try_convs(2)
                try_convs(1000)

                if debug_outs:
                    nc.sync.dma_start(out=dbg_dht_d.ap(), in_=dht_pad[:])
                    nc.sync.dma_start(out=dbg_h2_d.ap(), in_=h2_pad[:])

        for _rep in range(repeat):
            emit_pipeline()

    if split_waits:
        _split_multi_waits(nc, mybir)
    _PROGRAM_CACHE[key] = nc
    return nc


# ----------------------------------------------------------------------------
# entry point
# ----------------------------------------------------------------------------

def make_in_maps(inputs):
    T = _dht_tables()
    x = np.asarray(inputs['x'], np.float32)
    w1p, bias1, w2p, bias2, w3p, bias3 = _prep_weights(
        *[np.asarray(inputs[k], np.float32) for k in
          ('w1', 'b1', 'g1', 'be1', 'm1', 'v1',
           'w2', 'b2', 'g2', 'be2', 'm2', 'v2',
           'w3', 'b3', 'g3', 'be3', 'm3', 'v3')])
    b1f = np.zeros((1, 1024), np.float32)
    b1f[0, :512] = np.tile(bias1[:, 0] * W1SCALE, 4)
    common = dict(w1p=w1p, bias1f8=b1f.astype(F8), w2p=w2p, bias2=bias2,
                  w3p=w3p, bias3=bias3, ohtable=T['ohtable'])
    return [
        {'x': np.ascontiguousarray(x[n]).reshape(CIN, HW).astype(F8),
         **common}
        for n in range(N)
    ]


def run(inputs, trace=False):
    from concourse.bass_utils import run_bass_kernel_spmd

    nc = _build_program()
    in_maps = make_in_maps(inputs)
    res = run_bass_kernel_spmd(nc, in_maps, core_ids=list(range(N)),
                               trace=trace)
    out = np.stack([res.results[n]['out'].reshape(CMID, H, W)
                    for n in range(N)], axis=0)
    return out.astype(np.float32), res


def kernel(**inputs):
    out, _ = run(inputs, trace=False)
    return out
